# revision 59
# baseline (speedup 1.0000x reference)
"""Approximate EMD loss (B=16, N=M=2048, D=3) on 8 TRN2 NeuronCores.

Data-parallel over batch: each core owns 2 batch items, processed
sequentially with the full [M, N] pre-shifted distance matrix resident in
SBUF (fp16, partition dim = M).

Per level s with temperature lv:
  U = exp(lv * d2s)                      ACT (prefetched into prior step)
  rd[n] = sum_m satr'*U                  PE colsum (lhsT = satr', bf16)
  a[n]  = satl[n] / rd[n]                tile-domain via DRAM-transpose dance
  V = U * satr' * a_b (accum -> ss[m])   one DVE scalar_tensor_tensor
  scale2 = min(satr/(ss+eps), 1)         tiny [128,16] ops
  s[n]  = sum_m scale2*V                 PE colsum (lhsT = scale2)  [= svec]
  T = V * d2s                            DVE/Pool tensor_tensor (split)
  zt[n] = sum_m scale2*T                 PE colsum
  loss += sum_n (shift*s + zt)           tiny (shift folds the d2s pre-shift)
  satl = relu(satl - s); satr = relu(satr - scale2*ss)
The match matrix is never materialized; zz never exists per-(m,n): the
a-weighting rides inside V so s and zt come straight off the PE. n-domain
vectors round-trip through DRAM scratch for the [1,2048] <-> [128,16]
transposes (SBUF APs cannot retag partition dims; DRAM APs can).
"""
import sys
import os
import numpy as np

sys.path.insert(0, "/opt/trn_rl_repo")

B, N, M, D = 16, 2048, 2048, 3
P = 128
NT = M // P          # 16 m-tiles (partition dim tiles)
NCCH = N // P        # 16 n-chunks for the transposed [128,16] domain
NCORES = 8
BPC = B // NCORES    # 2 batch items per core
LEVELS = np.arange(8, -3, -0.25).astype(np.float32)
LEVELS[-1] = 0.0
NSTEPS = len(LEVELS)  # 44
NEG_START = 33        # first step with negative level (d2s re-shift point)

U_BUFS = int(os.environ.get("EMD_U_BUFS", "16"))
T_DVE = int(os.environ.get("EMD_T_DVE", "2"))   # T tiles computed on DVE (rest Pool)
STT_DVE = int(os.environ.get("EMD_STT_DVE", "5"))  # V-pass tiles on DVE (rest Pool)
KERNEL_NSTEPS = int(os.environ.get("EMD_NSTEPS", str(NSTEPS)))

_CACHE = {}


def _build(nsteps, u_bufs=U_BUFS, t_dve=T_DVE, stt_dve=STT_DVE):
    import concourse.bacc as bacc
    import concourse.mybir as mybir
    import concourse.bass_isa as bass_isa
    from concourse.tile import TileContext
    from concourse.alu_op_type import AluOpType

    dtf = mybir.dt.float32
    dtl = mybir.dt.bfloat16
    dth = mybir.dt.float16
    AF = mybir.ActivationFunctionType
    MUL = AluOpType.mult
    ADD = AluOpType.add

    dve_T = set(round(k * NT / t_dve) for k in range(t_dve)) if t_dve > 0 else set()
    dve_stt = set(round(k * NT / stt_dve) for k in range(stt_dve)) if stt_dve > 0 else set()

    nc = bacc.Bacc(None, target_bir_lowering=False)
    # xt pre-scaled by -2 on the host; yt plain transpose.
    xt_d = nc.dram_tensor("xt", [BPC, D, N], dtl, kind="ExternalInput")
    yt_d = nc.dram_tensor("yt", [BPC, D, M], dtl, kind="ExternalInput")
    ynt_d = nc.dram_tensor("ynt", [BPC, P, NT], dtf, kind="ExternalInput")   # y-norms, m-major
    xadj_d = nc.dram_tensor("xadj", [BPC, 1, N], dtf, kind="ExternalInput")  # xn - rowmax
    rdif_d = nc.dram_tensor("rdif", [BPC, 1, N], dtf, kind="ExternalInput")  # rowmax - rowmin
    rmax_d = nc.dram_tensor("rmax", [BPC, P, NCCH], dtf, kind="ExternalInput")  # n-dom
    rmin_d = nc.dram_tensor("rmin", [BPC, P, NCCH], dtf, kind="ExternalInput")
    out_d = nc.dram_tensor("out", [1, BPC], dtf, kind="ExternalOutput")

    with TileContext(nc) as tc:
        with tc.tile_pool(name="d2", bufs=1) as d2p, \
             tc.tile_pool(name="state", bufs=1) as stp:
            for b in range(BPC):
                d2_sb = d2p.tile([P, NT * N], dth, tag="d2")
                satl = stp.tile([P, NCCH], dtf, tag="satl")
                satr_t = stp.tile([P, NT], dtf, tag="satr")
                lossacc = stp.tile([P, NCCH], dtf, tag="lossacc")
                rmax_t = stp.tile([P, NCCH], dtf, tag="rmax")
                rmin_t = stp.tile([P, NCCH], dtf, tag="rmin")
                nc.vector.memset(satl[:], 1.0)
                nc.vector.memset(satr_t[:], 1.0)
                nc.vector.memset(lossacc[:], 0.0)
                nc.sync.dma_start(out=rmax_t[:], in_=rmax_d[b])
                nc.sync.dma_start(out=rmin_t[:], in_=rmin_d[b])

                # ---- setup: d2s = ((-2 y.x) + ym) + (xn - rowmax), fp16
                with tc.tile_pool(name="su", bufs=1) as su, \
                     tc.tile_pool(name="sups", bufs=4, space="PSUM") as sups:
                    xt_sb = su.tile([D, N], dtl, tag="xt")
                    yt_sb = su.tile([D, M], dtl, tag="yt")
                    ynt_sb = su.tile([P, NT], dtf, tag="ynt")
                    xadj_sb = su.tile([1, N], dtf, tag="xadj")
                    xadj_b = su.tile([P, N], dtf, tag="xadj_b")
                    nc.sync.dma_start(out=xt_sb[:], in_=xt_d[b])
                    nc.sync.dma_start(out=yt_sb[:], in_=yt_d[b])
                    nc.sync.dma_start(out=ynt_sb[:], in_=ynt_d[b])
                    nc.sync.dma_start(out=xadj_sb[:], in_=xadj_d[b])
                    nc.gpsimd.partition_broadcast(xadj_b[:], xadj_sb[0:1, :])
                    for i in range(NT):
                        for c in range(N // 512):
                            cps = sups.tile([P, 512], dtf, tag="cross")
                            nc.tensor.matmul(
                                cps[:], yt_sb[:, i * P:(i + 1) * P],
                                xt_sb[:, c * 512:(c + 1) * 512])
                            nc.vector.scalar_tensor_tensor(
                                d2_sb[:, i * N + c * 512: i * N + (c + 1) * 512],
                                cps[:], ynt_sb[:, i:i + 1],
                                xadj_b[:, c * 512:(c + 1) * 512], ADD, ADD)

                # ---- 44-level matching loop
                with tc.tile_pool(name="up", bufs=u_bufs) as up, \
                     tc.tile_pool(name="tp", bufs=4) as tp, \
                     tc.tile_pool(name="vs", bufs=4) as vs, \
                     tc.tile_pool(name="bb", bufs=2) as bb, \
                     tc.tile_pool(name="sm", bufs=2) as sm, \
                     tc.tile_pool(name="rows", bufs=1) as rows, \
                     tc.tile_pool(name="dscr", bufs=2, space="DRAM") as dscr, \
                     tc.tile_pool(name="rps", bufs=1, space="PSUM") as rps:
                    c_eps9 = stp.tile([P, NT], dtf, tag="c_eps9")
                    c_one = stp.tile([P, NT], dtf, tag="c_one")
                    c_zero = stp.tile([P, NT], dtf, tag="c_zero")
                    c_eps30 = stp.tile([P, NT], dtf, tag="c_eps30")
                    nc.vector.memset(c_eps9[:], 1e-9)
                    nc.vector.memset(c_one[:], 1.0)
                    nc.vector.memset(c_zero[:], 0.0)
                    nc.vector.memset(c_eps30[:], 1e-30)
                    # satr'(step 0) = 1.0 (bf16, rd-colsum stationary + stt scalar)
                    satr_lp_prev = sm.tile([P, NT], dtl, tag="satr_lp")
                    nc.vector.memset(satr_lp_prev[:], 1.0)
                    def emit_exp_rd(i, lv_, rd_ps_, satr_lp_, U_list):
                        # exp of tile i for a step at temperature lv_, plus its
                        # 4 rd-colsum matmuls (PSUM-accumulated over i)
                        U = up.tile([P, N], dtl, tag="U")
                        nc.scalar.activation(U[:], d2_sb[:, i * N:(i + 1) * N],
                                             AF.Exp, scale=lv_)
                        U_list.append(U)
                        for c in range(N // 512):
                            nc.tensor.matmul(
                                rd_ps_[0:1, c * 512:(c + 1) * 512],
                                satr_lp_[:, i:i + 1],
                                U[:, c * 512:(c + 1) * 512],
                                start=(i == 0), stop=(i == NT - 1))

                    U_cur = None
                    rd_ps_cur = None
                    pend = None  # (szt_t, shift_t) of step s-1, loss close deferred
                    for s in range(nsteps):
                        lv = float(LEVELS[s])
                        prefetch = (s + 1 < nsteps) and (s + 1 != NEG_START)
                        if s == NEG_START:
                            # re-shift d2s to d2 - rowmin for negative levels
                            rd_sb = rows.tile([1, N], dtf, tag="rd_row")
                            rd_bf = rows.tile([1, N], dth, tag="a_row")
                            rd_b = bb.tile([P, N], dth, tag="rs_b")
                            nc.sync.dma_start(out=rd_sb[:], in_=rdif_d[b])
                            nc.vector.tensor_copy(rd_bf[:], rd_sb[:])
                            nc.gpsimd.partition_broadcast(rd_b[:], rd_bf[0:1, :])
                            for i in range(NT):
                                nc.vector.tensor_tensor(
                                    d2_sb[:, i * N:(i + 1) * N],
                                    d2_sb[:, i * N:(i + 1) * N], rd_b[:], ADD)
                        if U_cur is None:
                            # cold start (s=0) or pipeline break (reshift)
                            rd_ps_cur = rps.tile([1, N], dtf, tag="rowps")
                            U_cur = []
                            for i in range(NT):
                                emit_exp_rd(i, lv, rd_ps_cur, satr_lp_prev, U_cur)
                        # rd -> a -> a_b  (n-domain dance), high prio: critical
                        with tc.high_priority(offset=120):
                            rd_row = rows.tile([1, N], dtf, tag="rd_row")
                            nc.scalar.copy(rd_row[:], rd_ps_cur[0:1, :])
                            scrA = dscr.tile([1, N], dtf, tag="scrA")
                            nc.scalar.dma_start(out=scrA[:], in_=rd_row[:])
                            rd_t = sm.tile([P, NCCH], dtf, tag="rd_t")
                            nc.scalar.dma_start(
                                out=rd_t[:],
                                in_=scrA[0:1, :].rearrange("o (p c) -> p (o c)", p=P))
                            rinv = sm.tile([P, NCCH], dtf, tag="rinv")
                            nc.vector.reciprocal(rinv[:], rd_t[:])
                            a_lp = sm.tile([P, NCCH], dtl, tag="a_lp")
                            nc.vector.tensor_tensor(a_lp[:], rinv[:], satl[:], MUL)
                            scrB = dscr.tile([1, N], dtl, tag="scrB")
                            a_row = rows.tile([1, N], dtl, tag="a_row")
                            a_b = bb.tile([P, N], dtl, tag="a_b")
                            for cch in range(4):
                                cs, ce = cch * 512, (cch + 1) * 512
                                nc.sync.dma_start(
                                    out=scrB[0:1, cs:ce].rearrange(
                                        "o (p c) -> p (o c)", p=32),
                                    in_=a_lp[32 * cch:32 * (cch + 1), :])
                                nc.sync.dma_start(out=a_row[0:1, cs:ce],
                                                  in_=scrB[0:1, cs:ce])
                                nc.gpsimd.partition_broadcast(
                                    a_b[:, cs:ce], a_row[0:1, cs:ce])
                        # deferred loss close of step s-1 (off critical path)
                        if pend is not None:
                            zt_t_p, svec_t_p, shift_p = pend
                            corr = sm.tile([P, NCCH], dtf, tag="corr")
                            nc.vector.tensor_tensor(corr[:], shift_p[:],
                                                    svec_t_p[:], MUL)
                            nc.vector.tensor_add(corr[:], corr[:], zt_t_p[:])
                            nc.vector.tensor_add(lossacc[:], lossacc[:], corr[:])
                            pend = None
                        # C: V (+ss), scale2, T, svec/zt colsums, satr' update,
                        # interleaved with next step's exp+rd prefetch
                        ss = sm.tile([P, NT], dtf, tag="ss")
                        szt_ps = rps.tile([33, N], dtf, tag="szt")
                        sc2_lp = sm.tile([P, NT], dth, tag="sc2")
                        ssp_ = sm.tile([P, NT], dtf, tag="ssp")
                        sinv = sm.tile([P, NT], dtf, tag="sinv")
                        scale2 = sm.tile([P, NT], dtf, tag="scale2")
                        ss2 = sm.tile([P, NT], dtf, tag="ss2")
                        satrp = sm.tile([P, NT], dtf, tag="satrp")
                        satr_lp = sm.tile([P, NT], dtl, tag="satr_lp")
                        if prefetch:
                            rd_ps_nxt = rps.tile([1, N], dtf, tag="rowps")
                            U_nxt = []
                            nxt_lv = float(LEVELS[s + 1])
                        # group boundaries: singletons first so scale2(g0) is
                        # ready after one stt, unblocking the PE sooner
                        BOUNDS = [0, 1, 2, 3, 4, 6, 8, 10, 12, 14, 16]
                        DEFER = NT - 4  # tiles >= DEFER: T+zt run after phase D
                        V_defer = {}
                        for g in range(len(BOUNDS) - 1):
                            gs, ge = BOUNDS[g], BOUNDS[g + 1]
                            Vg = []
                            for i in range(gs, ge):
                                V = vs.tile([P, N], dth, tag="V")
                                if i == 0:
                                    ss_part = sm.tile([P, 4], dtf, tag="ss_part")
                                    for cch in range(4):
                                        cs, ce = cch * 512, (cch + 1) * 512
                                        nc.vector.scalar_tensor_tensor(
                                            V[:, cs:ce], U_cur[i][:, cs:ce],
                                            satr_lp_prev[:, i:i + 1],
                                            a_b[:, cs:ce], MUL, MUL,
                                            accum_out=ss_part[:, cch:cch + 1])
                                    with tc.high_priority(offset=40):
                                        nc.vector.reduce_sum(
                                            ss[:, 0:1], ss_part[:],
                                            axis=mybir.AxisListType.X)
                                else:
                                    nc.vector.scalar_tensor_tensor(
                                        V[:], U_cur[i][:], satr_lp_prev[:, i:i + 1],
                                        a_b[:], MUL, MUL,
                                        accum_out=ss[:, i:i + 1])
                                Vg.append(V)
                            # scale2 for this group (high prio: unblocks PE)
                            with tc.high_priority(offset=40):
                                nc.vector.tensor_scalar(ssp_[:, gs:ge], ss[:, gs:ge],
                                                        1e-9, None, ADD)
                                nc.vector.reciprocal(sinv[:, gs:ge], ssp_[:, gs:ge])
                                nc.vector.tensor_tensor(scale2[:, gs:ge],
                                                        sinv[:, gs:ge],
                                                        satr_t[:, gs:ge], MUL)
                                nc.vector.tensor_scalar(scale2[:, gs:ge],
                                                        scale2[:, gs:ge],
                                                        1.0, None, AluOpType.min)
                                nc.vector.tensor_copy(sc2_lp[:, gs:ge],
                                                      scale2[:, gs:ge])
                            for i in range(gs, ge):
                                V = Vg[i - gs]
                                for c in range(N // 512):
                                    nc.tensor.matmul(
                                        szt_ps[0:1, c * 512:(c + 1) * 512],
                                        sc2_lp[:, i:i + 1],
                                        V[:, c * 512:(c + 1) * 512],
                                        start=(i == 0), stop=(i == NT - 1))
                                if i >= DEFER:
                                    V_defer[i] = V
                                    continue
                                T = tp.tile([P, N], dth, tag="T")
                                eng = nc.vector if i in dve_T else nc.gpsimd
                                eng.tensor_tensor(T[:], V[:],
                                                  d2_sb[:, i * N:(i + 1) * N], MUL)
                                for c in range(N // 512):
                                    nc.tensor.matmul(
                                        szt_ps[32:33, c * 512:(c + 1) * 512],
                                        sc2_lp[:, i:i + 1],
                                        T[:, c * 512:(c + 1) * 512],
                                        start=(i == 0), stop=(i == NT - 1))
                            # group satr' update (unlocks next step's rd weights)
                            with tc.high_priority(offset=40):
                                nc.vector.tensor_tensor(ss2[:, gs:ge], ss[:, gs:ge],
                                                        scale2[:, gs:ge], MUL)
                                nc.vector.tensor_sub(satr_t[:, gs:ge],
                                                     satr_t[:, gs:ge],
                                                     ss2[:, gs:ge])
                                nc.vector.tensor_scalar(satr_t[:, gs:ge],
                                                        satr_t[:, gs:ge],
                                                        0.0, None, AluOpType.max)
                                nc.vector.tensor_scalar(satr_lp[:, gs:ge],
                                                        satr_t[:, gs:ge],
                                                        1e-30, None, ADD)
                            if prefetch:
                                for i in range(gs, ge):
                                    emit_exp_rd(i, nxt_lv, rd_ps_nxt, satr_lp, U_nxt)
                        # n-domain: svec/zt dance + satl update (critical path)
                        with tc.high_priority(offset=60):
                            svec_row = rows.tile([1, N], dtf, tag="svec_row")
                            nc.scalar.copy(svec_row[:], szt_ps[0:1, :])
                            scrC = dscr.tile([1, N], dtf, tag="scrC")
                            nc.sync.dma_start(out=scrC[:], in_=svec_row[:])
                            svec_t = sm.tile([P, NCCH], dtf, tag="svec_t")
                            nc.sync.dma_start(
                                out=svec_t[:],
                                in_=scrC[0:1, :].rearrange("o (p c) -> p (o c)", p=P))
                            nc.vector.tensor_sub(satl[:], satl[:], svec_t[:])
                            nc.vector.tensor_scalar(satl[:], satl[:], 0.0, None,
                                                    AluOpType.max)
                        for i in sorted(V_defer):
                            T = tp.tile([P, N], dth, tag="T")
                            eng = nc.vector if i in dve_T else nc.gpsimd
                            eng.tensor_tensor(T[:], V_defer[i][:],
                                              d2_sb[:, i * N:(i + 1) * N], MUL)
                            for c in range(N // 512):
                                nc.tensor.matmul(
                                    szt_ps[32:33, c * 512:(c + 1) * 512],
                                    sc2_lp[:, i:i + 1],
                                    T[:, c * 512:(c + 1) * 512],
                                    start=(i == 0), stop=(i == NT - 1))
                        zt_row = rows.tile([1, N], dtf, tag="zt_row")
                        nc.scalar.copy(zt_row[:], szt_ps[32:33, :])
                        scrC2 = dscr.tile([1, N], dtf, tag="scrC2")
                        nc.scalar.dma_start(out=scrC2[:], in_=zt_row[:])
                        zt_t = sm.tile([P, NCCH], dtf, tag="zt_t")
                        nc.scalar.dma_start(
                            out=zt_t[:],
                            in_=scrC2[0:1, :].rearrange("o (p c) -> p (o c)", p=P))
                        pend = (zt_t, svec_t, rmax_t if s < NEG_START else rmin_t)
                        satr_lp_prev = satr_lp
                        if prefetch:
                            U_cur = U_nxt
                            rd_ps_cur = rd_ps_nxt
                        else:
                            U_cur = None
                            rd_ps_cur = None
                    # close the last step's loss contribution
                    zt_t_p, svec_t_p, shift_p = pend
                    corr = sm.tile([P, NCCH], dtf, tag="corr")
                    nc.vector.tensor_tensor(corr[:], shift_p[:], svec_t_p[:], MUL)
                    nc.vector.tensor_add(corr[:], corr[:], zt_t_p[:])
                    nc.vector.tensor_add(lossacc[:], lossacc[:], corr[:])
                    # final per-batch reduction
                    lsum = sm.tile([P, 1], dtf, tag="lsum")
                    nc.vector.reduce_sum(lsum[:], lossacc[:], axis=mybir.AxisListType.X)
                    lall = sm.tile([P, 1], dtf, tag="lall")
                    nc.gpsimd.partition_all_reduce(
                        lall[:], lsum[:], channels=P, reduce_op=bass_isa.ReduceOp.add)
                    nc.sync.dma_start(out=out_d[0:1, b:b + 1], in_=lall[0:1, 0:1])
    nc.finalize()
    return nc


def _host_prep(points_x, points_y, nsteps):
    px = np.ascontiguousarray(points_x, np.float32)
    py = np.ascontiguousarray(points_y, np.float32)
    in_maps = []
    for core in range(NCORES):
        import ml_dtypes
        xt = np.empty((BPC, D, N), ml_dtypes.bfloat16)
        yt = np.empty((BPC, D, M), ml_dtypes.bfloat16)
        ynt = np.empty((BPC, P, NT), np.float32)
        xadj = np.empty((BPC, 1, N), np.float32)
        rdif = np.empty((BPC, 1, N), np.float32)
        rmax = np.empty((BPC, P, NCCH), np.float32)
        rmin = np.empty((BPC, P, NCCH), np.float32)
        for b in range(BPC):
            gb = core * BPC + b
            x, y = px[gb], py[gb]
            xt[b] = (-2.0 * x).T
            yt[b] = y.T
            xnv = (x * x).sum(-1).astype(np.float32)          # [N]
            ynv = (y * y).sum(-1).astype(np.float32)          # [M]
            ynt[b] = ynv.reshape(NT, P).T
            # d2 in the same assoc order the device uses: (cross + ym) + (xn - rowmax)
            d2 = ((-2.0 * x) @ y.T).T + ynv[:, None] + xnv[None, :]   # [M, N]
            rowmax = d2.max(0)   # over m, per n
            rowmin = d2.min(0)
            xadj[b, 0] = xnv - rowmax
            rdif[b, 0] = rowmax - rowmin
            rmax[b] = rowmax.reshape(P, NCCH)
            rmin[b] = rowmin.reshape(P, NCCH)
        in_maps.append({"xt": xt, "yt": yt, "ynt": ynt, "xadj": xadj,
                        "rdif": rdif, "rmax": rmax, "rmin": rmin})
    return in_maps


def _get_built(nsteps):
    key = (nsteps, U_BUFS, T_DVE)
    if key not in _CACHE:
        _CACHE[key] = _build(nsteps)
    return _CACHE[key]


def run(points_x, points_y, nsteps=None, trace=False):
    from concourse.bass_utils import run_bass_kernel_spmd
    nsteps = nsteps or KERNEL_NSTEPS
    nc = _get_built(nsteps)
    in_maps = _host_prep(points_x, points_y, nsteps)
    res = run_bass_kernel_spmd(nc, in_maps, core_ids=list(range(NCORES)),
                               trace=trace)
    outs = np.array([res.results[c]["out"].reshape(-1) for c in range(NCORES)])
    loss = np.float32(outs.sum() / np.float32(B))
    return loss, res


def kernel(points_x, points_y):
    loss, _ = run(points_x, points_y)
    return np.asarray(loss, np.float32)
